# revision 1
# baseline (speedup 1.0000x reference)
"""Trainium2 Bass kernel v2 for nn_AngularDescriptor (gnn_message_passing).

Legendre-addition-theorem factorization (q = 0.5*(sum_m A^2 - B)) over
N*M pairs.  Gather strategy: u16-packed position table replicated to all
128 SBUF partitions, per-pair records fetched with GPSIMD ap_gather
(16-partition-group shared index lists), per-partition slices extracted
with partition-strided SBUF->SBUF DMAs, converted u16->f32 on the ACT
engine.  Types / c_table rows are host-marshaled (index lookups only).
Compute: DVE pipeline with bf16 2x tensor-tensor ops and tree
reductions; gathers/extracts overlap compute via 2-slot chunks.
"""
import os
import sys

sys.path.insert(0, "/opt/trn_rl_repo")
os.environ.setdefault("NEURON_RT_RESET_CORES", "1")

import math
import numpy as np

from concourse import bacc, bass, mybir, tile, library_config
from concourse.bass_utils import run_bass_kernel_spmd

# problem constants
N_ATOMS = 10000
M_NBR = 20
N_TYPES = 4
N_DESC = 8
K_MAX = 8
L_MAX = 4
R_C = 5.0

NCORES = 8
P = 128
S = 10                      # atom slots per partition
CA = P * S                  # atoms per core = 1280
NTOT = NCORES * CA          # padded atom count = 10240
PAIRS = S * M_NBR           # 200 pairs per partition
D = 4                       # u16 per table record (x, y, z, pad)
POS_SCALE = 16384.0

SLOT_CHUNKS = [2, 2, 2, 2, 2]        # slots per chunk (sum = S)
NCH = len(SLOT_CHUNKS)

F32 = mybir.dt.float32
BF16 = mybir.dt.bfloat16
U16 = mybir.dt.uint16
I16 = mybir.dt.int16

SQ3 = math.sqrt(3.0)
C31 = math.sqrt(3.0 / 8.0)
C32 = math.sqrt(15.0)
C33 = math.sqrt(5.0 / 8.0)
SHELL_OFF = [0, 1, 4, 9, 16]

AF = mybir.ActivationFunctionType
ALU = mybir.AluOpType
AX = mybir.AxisListType


def _ap(t, off, dims):
    base = t[:]
    ap = [list(base.ap[0])] + [[s, c] for (s, c) in dims]
    return bass.AP(base.tensor, base.offset + off, ap)


def _aps(t, r, off, dims):
    """Partitions r::16, free offset off."""
    base = t[r::16]
    ap = [list(base.ap[0])] + [[s, c] for (s, c) in dims]
    return bass.AP(base.tensor, base.offset + off, ap)


def build_nc(debug=False):
    nc = bacc.Bacc()
    tbl_d = nc.declare_dram_parameter("tbl", [P, NTOT * D], U16, isOutput=False)
    ctr_d = nc.declare_dram_parameter("ctr", [P, S * 3], F32, isOutput=False)
    cj_d = nc.declare_dram_parameter("cj", [P, PAIRS * 64], BF16,
                                     isOutput=False)
    gi_d = nc.declare_dram_parameter("gi", [P, PAIRS], I16, isOutput=False)
    out_d = nc.declare_dram_parameter("out", [P, S * N_DESC * L_MAX], F32,
                                      isOutput=True)

    with tile.TileContext(nc) as tc:
        with tc.tile_pool(name="main", bufs=1) as pool:
            tbl = pool.tile([P, NTOT * D], U16)
            third = NTOT * D // 4 * 2   # keep 4-elem alignment
            six = NTOT * D // 4
            nc.gpsimd.dma_start(out=tbl[:, six:six + six],
                                in_=tbl_d[:, six:six + six])
            nc.gpsimd.load_library(library_config.ap_gather)

            cvals = [math.pi / 2, 0.25, -1.0, -0.5, -1.5,
                     -C31]
            consts = pool.tile([P, len(cvals)], F32)
            for ci, cv in enumerate(cvals):
                nc.vector.memset(consts[:, ci:ci + 1], cv)
                nc.const_aps.aps[(F32, cv)] = consts[:, ci:ci + 1]

            # small inputs FIRST so the first gather isn't queued behind
            # the 10MB table load
            ctr = pool.tile([P, S * 3], F32)
            gi = pool.tile([P, PAIRS], I16)
            nc.sync.dma_start(out=gi[:], in_=gi_d[:])
            nc.sync.dma_start(out=ctr[:], in_=ctr_d[:])

            nc.sync.dma_start(out=tbl[:, :six], in_=tbl_d[:, :six])
            nc.scalar.dma_start(out=tbl[:, six + six:],
                                in_=tbl_d[:, six + six:])

            # persistent accumulators
            A = pool.tile([P, S * N_DESC * 16], F32)     # [s, d, m]
            g_all = pool.tile([P, PAIRS * N_DESC], BF16)  # [s, j, d]
            Bh = pool.tile([P, S * N_DESC], F32)          # [s, d]

            with tc.tile_pool(name="gath", bufs=3) as gp, \
                    tc.tile_pool(name="chunk", bufs=2) as cp:
                s_base = 0
                for ch, SL in enumerate(SLOT_CHUNKS):
                    CH = SL * M_NBR           # pairs per partition this chunk
                    NI = 16 * CH              # gathered records per group
                    dbg = {} if (debug and ch == 0) else None
                    build_chunk(nc, tc, gp, cp, ch, SL, CH, NI, s_base,
                                tbl, gi, ctr, cj_d, A, g_all, Bh,
                                dbg=dbg)
                    if dbg is not None:
                        for nm, (t, dt) in dbg.items():
                            dd = nc.declare_dram_parameter(
                                "d_" + nm, [P, t.shape[1]], dt, isOutput=True)
                            nc.sync.dma_start(out=dd[:], in_=t[:])
                    s_base += SL
            if debug:
                for nm, t, dt in [("g_all", g_all, BF16), ("A", A, F32),
                                  ("Bh", Bh, F32)]:
                    dd = nc.declare_dram_parameter(
                        "d_" + nm, [P, t.shape[1]], dt, isOutput=True)
                    nc.sync.dma_start(out=dd[:], in_=t[:])

            # g is sqrt(0.5)-scaled via call, so q = sum_shell A^2 - B
            Asq = pool.tile([P, S * N_DESC * 16], F32)
            nc.vector.tensor_tensor(out=Asq[:], in0=A[:], in1=A[:],
                                    op=ALU.mult)
            outq = pool.tile([P, S * N_DESC * L_MAX], F32)
            q2l = pool.tile([P, S * N_DESC], F32)
            for l in range(L_MAX):
                cnt = SHELL_OFF[l + 1] - SHELL_OFF[l]
                nc.vector.tensor_reduce(
                    out=q2l[:],
                    in_=_ap(Asq, SHELL_OFF[l],
                            [(N_DESC * 16, S), (16, N_DESC), (1, cnt)]),
                    axis=AX.X, op=ALU.add)
                nc.vector.tensor_tensor(
                    out=_ap(outq, l, [(N_DESC * L_MAX, S), (L_MAX, N_DESC)]),
                    in0=q2l[:], in1=Bh[:], op=ALU.subtract)

            nc.sync.dma_start(out=out_d[:], in_=outq[:])
    nc.finalize()
    return nc


def build_chunk(nc, tc, gp, cp, ch, SL, CH, NI, s_base,
                tbl, gi, ctr, cj_d, A, g_all, Bh, dbg=None):
    """One pipeline chunk: SL atom slots (CH pairs/partition)."""
    col0 = s_base * M_NBR             # idx cols consumed by prior chunks
    # idx columns for this chunk: NI//16 = CH
    gout = gp.tile([P, NI * D], U16, tag="gout", name="gout")
    nc.gpsimd.ap_gather(
        _ap(gout, 0, [(D, NI), (1, D)]),
        _ap(tbl, 0, [(D, NTOT), (1, D)]),
        gi[:, col0:col0 + CH], channels=P, num_elems=NTOT, d=D, num_idxs=NI)

    # extract per-partition slice (records [r*CH, r*CH+CH) for r = p%16)
    gsel = gp.tile([P, CH * D], U16, tag="gsel", name="gsel")
    for r in range(16):
        nc.sync.dma_start(out=_aps(gsel, r, 0, [(1, CH * D)]),
                          in_=_aps(gout, r, r * CH * D, [(1, CH * D)]))

    # u16 -> f32 (drop pad component), ACT engine
    posj = gp.tile([P, CH * 3], F32, tag="posj", name="posj")
    nc.scalar.activation(out=_ap(posj, 0, [(3, CH), (1, 3)]),
                         in_=_ap(gsel, 0, [(D, CH), (1, 3)]),
                         func=AF.Copy, scale=1.0 / POS_SCALE)
    if dbg is not None:
        dbg["gout"] = (gout, U16)
        dbg["gsel"] = (gsel, U16)
        dbg["posj"] = (posj, F32)

    # ---- geometry --------------------------------------------------------
    C3 = CH * 3
    dxyz = cp.tile([P, C3], F32, tag="dxyz", name="dxyz")
    nc.vector.tensor_tensor(
        out=dxyz[:], in0=posj[:],
        in1=_ap(ctr, s_base * 3, [(3, SL), (0, M_NBR), (1, 3)]),
        op=ALU.subtract)
    sq = cp.tile([P, C3], F32, tag="sq", name="sq")
    nc.vector.tensor_tensor(out=sq[:], in0=dxyz[:], in1=dxyz[:], op=ALU.mult)
    r2 = cp.tile([P, CH], F32, tag="r2", name="r2")
    nc.vector.tensor_reduce(out=r2[:], in_=_ap(sq, 0, [(3, CH), (1, 3)]),
                            axis=AX.X, op=ALU.add)
    rr = cp.tile([P, CH], F32, tag="rr", name="rr")
    nc.scalar.sqrt(out=rr[:], in_=r2[:])
    rinv = cp.tile([P, CH], F32, tag="rinv", name="rinv")
    nc.vector.reciprocal(out=rinv[:], in_=rr[:])
    u = cp.tile([P, C3], F32, tag="u", name="u")
    nc.vector.tensor_tensor(out=u[:], in0=dxyz[:],
                            in1=_ap(rinv, 0, [(1, CH), (0, 3)]),
                            op=ALU.mult)

    # ---- Chebyshev radial basis -----------------------------------------
    s01 = cp.tile([P, CH], F32, tag="s01", name="s01")
    nc.vector.tensor_scalar_mul(out=s01[:], in0=rr[:], scalar1=1.0 / R_C)
    cosx = cp.tile([P, CH], F32, tag="cosx", name="cosx")
    nc.scalar.activation(out=cosx[:], in_=s01[:], func=AF.Sin,
                         bias=math.pi / 2, scale=-math.pi)
    mask = cp.tile([P, CH], F32, tag="mask", name="mask")
    nc.vector.tensor_scalar(out=mask[:], in0=rr[:], scalar1=R_C, scalar2=None,
                            op0=ALU.is_lt)
    fch = cp.tile([P, CH], F32, tag="fch", name="fch")
    tmp0 = cp.tile([P, CH], F32, tag="tmp0", name="tmp0")
    nc.vector.tensor_scalar(out=tmp0[:], in0=cosx[:], scalar1=0.25,
                            scalar2=0.25, op0=ALU.mult, op1=ALU.add)
    nc.vector.tensor_tensor(out=fch[:], in0=tmp0[:], in1=mask[:], op=ALU.mult)
    tm1 = cp.tile([P, CH], F32, tag="tm1", name="tm1")
    nc.vector.tensor_scalar(out=tm1[:], in0=s01[:], scalar1=-1.0, scalar2=None,
                            op0=ALU.add)
    xc = cp.tile([P, CH], F32, tag="xc", name="xc")
    nc.vector.scalar_tensor_tensor(out=xc[:], in0=tm1[:], scalar=2.0,
                                   in1=tm1[:], op0=ALU.mult, op1=ALU.mult)
    nc.vector.tensor_scalar(out=xc[:], in0=xc[:], scalar1=-1.0, scalar2=None,
                            op0=ALU.add)
    x2 = cp.tile([P, CH], F32, tag="x2", name="x2")
    nc.vector.tensor_scalar_mul(out=x2[:], in0=xc[:], scalar1=2.0)

    f = cp.tile([P, CH * K_MAX], BF16, tag="f", name="f")  # [sl, j, k]

    def f_slice(k):
        return _ap(f, k, [(M_NBR * K_MAX, SL), (K_MAX, M_NBR)])

    with nc.allow_low_precision(reason="bf16 radial basis"):
        nc.vector.tensor_scalar_mul(out=f_slice(0), in0=fch[:], scalar1=2.0)
        nc.vector.scalar_tensor_tensor(out=f_slice(1), in0=xc[:], scalar=1.0,
                                       in1=fch[:], op0=ALU.add, op1=ALU.mult)
        Ta = cp.tile([P, CH], F32, tag="Ta", name="Ta")
        nc.vector.scalar_tensor_tensor(out=Ta[:], in0=xc[:], scalar=2.0,
                                       in1=xc[:], op0=ALU.mult, op1=ALU.mult)
        nc.vector.tensor_scalar(out=Ta[:], in0=Ta[:], scalar1=-1.0,
                                scalar2=None, op0=ALU.add)
        nc.vector.scalar_tensor_tensor(out=f_slice(2), in0=Ta[:], scalar=1.0,
                                       in1=fch[:], op0=ALU.add, op1=ALU.mult)
        Tprev, Tprev2 = Ta, xc
        for k in range(3, K_MAX):
            Tk = cp.tile([P, CH], F32, tag=f"cheb{k}", name=f"cheb{k}")
            nc.vector.tensor_tensor(out=Tk[:], in0=x2[:], in1=Tprev[:],
                                    op=ALU.mult)
            nc.vector.tensor_tensor(out=Tk[:], in0=Tk[:], in1=Tprev2[:],
                                    op=ALU.subtract)
            nc.vector.scalar_tensor_tensor(out=f_slice(k), in0=Tk[:],
                                           scalar=1.0, in1=fch[:],
                                           op0=ALU.add, op1=ALU.mult)
            Tprev, Tprev2 = Tk, Tprev

    # ---- spherical harmonics Y[16] (bf16) -------------------------------
    Y = cp.tile([P, CH * 16], BF16, tag="Y", name="Y")  # [sl, m, j]

    def y_slice(m, cnt=1):
        if cnt == 1:
            return _ap(Y, m * M_NBR, [(16 * M_NBR, SL), (1, M_NBR)])
        return _ap(Y, m * M_NBR,
                   [(16 * M_NBR, SL), (M_NBR, cnt), (1, M_NBR)])

    def u_c(c):
        return _ap(u, c, [(3 * M_NBR, SL), (3, M_NBR)])

    with nc.allow_low_precision(reason="bf16 Y"):
        nc.vector.memset(y_slice(0), 1.0)
        nc.vector.tensor_copy(out=y_slice(1, 3),
                              in_=_ap(u, 0, [(3 * M_NBR, SL), (1, 3),
                                             (3, M_NBR)]))
        x2c = cp.tile([P, CH], F32, tag="x2c", name="x2c")
        y2c = cp.tile([P, CH], F32, tag="y2c", name="y2c")
        z2c = cp.tile([P, CH], F32, tag="z2c", name="z2c")
        nc.vector.tensor_tensor(out=x2c[:], in0=u_c(0), in1=u_c(0),
                                op=ALU.mult)
        nc.vector.tensor_tensor(out=y2c[:], in0=u_c(1), in1=u_c(1),
                                op=ALU.mult)
        nc.vector.tensor_tensor(out=z2c[:], in0=u_c(2), in1=u_c(2),
                                op=ALU.mult)
        xyc = cp.tile([P, CH], F32, tag="xyc", name="xyc")
        nc.vector.tensor_tensor(out=xyc[:], in0=u_c(0), in1=u_c(1),
                                op=ALU.mult)
        nc.vector.tensor_scalar_mul(out=y_slice(4), in0=xyc[:], scalar1=SQ3)
        nc.vector.scalar_tensor_tensor(out=y_slice(5), in0=u_c(1), scalar=SQ3,
                                       in1=u_c(2), op0=ALU.mult, op1=ALU.mult)
        nc.vector.scalar_tensor_tensor(out=y_slice(6), in0=u_c(0), scalar=SQ3,
                                       in1=u_c(2), op0=ALU.mult, op1=ALU.mult)
        nc.vector.tensor_scalar(out=y_slice(7), in0=z2c[:], scalar1=1.5,
                                scalar2=-0.5, op0=ALU.mult, op1=ALU.add)
        dxyc = cp.tile([P, CH], F32, tag="dxyc", name="dxyc")
        nc.vector.tensor_tensor(out=dxyc[:], in0=x2c[:], in1=y2c[:],
                                op=ALU.subtract)
        nc.vector.tensor_scalar_mul(out=y_slice(8), in0=dxyc[:],
                                    scalar1=SQ3 / 2)
        tl3 = cp.tile([P, CH], F32, tag="tl3", name="tl3")
        nc.vector.tensor_scalar(out=tl3[:], in0=z2c[:], scalar1=2.5,
                                scalar2=-1.5, op0=ALU.mult, op1=ALU.add)
        nc.vector.tensor_tensor(out=y_slice(9), in0=tl3[:], in1=u_c(2),
                                op=ALU.mult)
        tl4 = cp.tile([P, CH], F32, tag="tl4", name="tl4")
        nc.vector.tensor_scalar(out=tl4[:], in0=z2c[:], scalar1=5.0 * C31,
                                scalar2=-C31, op0=ALU.mult, op1=ALU.add)
        nc.vector.tensor_tensor(out=y_slice(10), in0=tl4[:], in1=u_c(0),
                                op=ALU.mult)
        nc.vector.tensor_tensor(out=y_slice(11), in0=tl4[:], in1=u_c(1),
                                op=ALU.mult)
        nc.vector.scalar_tensor_tensor(out=y_slice(12), in0=dxyc[:],
                                       scalar=C32 / 2, in1=u_c(2),
                                       op0=ALU.mult, op1=ALU.mult)
        nc.vector.scalar_tensor_tensor(out=y_slice(13), in0=xyc[:],
                                       scalar=C32, in1=u_c(2),
                                       op0=ALU.mult, op1=ALU.mult)
        tl5 = cp.tile([P, CH], F32, tag="tl5", name="tl5")
        nc.vector.scalar_tensor_tensor(out=tl5[:], in0=y2c[:], scalar=3.0,
                                       in1=x2c[:], op0=ALU.mult,
                                       op1=ALU.subtract)
        nc.vector.scalar_tensor_tensor(out=y_slice(14), in0=tl5[:],
                                       scalar=-C33, in1=u_c(0),
                                       op0=ALU.mult, op1=ALU.mult)
        tl6 = cp.tile([P, CH], F32, tag="tl6", name="tl6")
        nc.vector.scalar_tensor_tensor(out=tl6[:], in0=x2c[:], scalar=3.0,
                                       in1=y2c[:], op0=ALU.mult,
                                       op1=ALU.subtract)
        nc.vector.scalar_tensor_tensor(out=y_slice(15), in0=tl6[:],
                                       scalar=C33, in1=u_c(1),
                                       op0=ALU.mult, op1=ALU.mult)

    if dbg is not None:
        dbg["rr"] = (rr, F32)
        dbg["f"] = (f, BF16)
        dbg["Y"] = (Y, BF16)

    # per-pair c rows c_table[t_i, t_j] in [j, d, k] order (host-built)
    cjt = cp.tile([P, CH * 64], BF16, tag="cjt", name="cjt")
    nc.scalar.dma_start(out=cjt[:],
                        in_=cj_d[:, s_base * M_NBR * 64:
                                 (s_base + SL) * M_NBR * 64])

    # ---- per-slot contractions ------------------------------------------
    for sl in range(SL):
        s = s_base + sl
        x2t = cp.tile([P, M_NBR * 64], BF16, tag="x2t", name="x2t")
        with nc.allow_low_precision(reason="bf16 contraction"):
            nc.vector.tensor_tensor(
                out=_ap(x2t, 0, [(64, M_NBR), (8, N_DESC), (1, 8)]),
                in0=_ap(cjt, sl * M_NBR * 64,
                        [(64, M_NBR), (8, N_DESC), (1, 8)]),
                in1=_ap(f, sl * M_NBR * K_MAX,
                        [(K_MAX, M_NBR), (0, N_DESC), (1, 8)]),
                op=ALU.mult)
            # tree reduce over k (8)
            a4 = cp.tile([P, M_NBR * 32], BF16, tag="a4", name="a4")
            nc.vector.tensor_tensor(
                out=_ap(a4, 0, [(32, M_NBR), (4, N_DESC), (1, 4)]),
                in0=_ap(x2t, 0, [(64, M_NBR), (8, N_DESC), (1, 4)]),
                in1=_ap(x2t, 4, [(64, M_NBR), (8, N_DESC), (1, 4)]),
                op=ALU.add)
            a2 = cp.tile([P, M_NBR * 16], BF16, tag="a2", name="a2")
            nc.vector.tensor_tensor(
                out=_ap(a2, 0, [(16, M_NBR), (2, N_DESC), (1, 2)]),
                in0=_ap(a4, 0, [(32, M_NBR), (4, N_DESC), (1, 2)]),
                in1=_ap(a4, 2, [(32, M_NBR), (4, N_DESC), (1, 2)]),
                op=ALU.add)
            # g[s,j,d] (bf16) into persistent g_all
            gsl = _ap(g_all, s * M_NBR * N_DESC,
                      [(1, M_NBR), (M_NBR, N_DESC)])
            nc.vector.tensor_tensor(
                out=gsl,
                in0=_ap(a2, 0, [(16, M_NBR), (2, N_DESC)]),
                in1=_ap(a2, 1, [(16, M_NBR), (2, N_DESC)]),
                op=ALU.add)

        # B[s,d] = sum_j g^2 (f32 out)
        gsq = cp.tile([P, M_NBR * N_DESC], F32, tag="gsq", name="gsq")
        nc.vector.tensor_tensor(
            out=gsq[:],
            in0=_ap(g_all, s * M_NBR * N_DESC, [(1, M_NBR * N_DESC)]),
            in1=_ap(g_all, s * M_NBR * N_DESC, [(1, M_NBR * N_DESC)]),
            op=ALU.mult)
        nc.vector.tensor_reduce(
            out=_ap(Bh, s * N_DESC, [(1, N_DESC)]),
            in_=_ap(gsq, 0, [(M_NBR, N_DESC), (1, M_NBR)]),
            axis=AX.X, op=ALU.add)

        # xa[j,d,m] = g x Y (bf16), tree over j, f32 finish -> A[s,d,m]
        xa = cp.tile([P, M_NBR * 128], BF16, tag="xa", name="xa")
        with nc.allow_low_precision(reason="bf16 outer product"):
            nc.vector.tensor_tensor(
                out=_ap(xa, 0, [(16 * M_NBR, N_DESC), (M_NBR, 16),
                                (1, M_NBR)]),
                in0=_ap(g_all, s * M_NBR * N_DESC,
                        [(M_NBR, N_DESC), (0, 16), (1, M_NBR)]),
                in1=_ap(Y, sl * M_NBR * 16,
                        [(0, N_DESC), (M_NBR, 16), (1, M_NBR)]),
                op=ALU.mult)
            t10 = cp.tile([P, 10 * 128], BF16, tag="t10", name="t10")
            nc.vector.tensor_tensor(
                out=_ap(t10, 0, [(160, N_DESC), (10, 16), (1, 10)]),
                in0=_ap(xa, 0, [(320, N_DESC), (20, 16), (1, 10)]),
                in1=_ap(xa, 10, [(320, N_DESC), (20, 16), (1, 10)]),
                op=ALU.add)
            t5 = cp.tile([P, 5 * 128], BF16, tag="t5", name="t5")
            nc.vector.tensor_tensor(
                out=_ap(t5, 0, [(80, N_DESC), (5, 16), (1, 5)]),
                in0=_ap(t10, 0, [(160, N_DESC), (10, 16), (1, 5)]),
                in1=_ap(t10, 5, [(160, N_DESC), (10, 16), (1, 5)]),
                op=ALU.add)
        nc.vector.tensor_reduce(
            out=_ap(A, s * N_DESC * 16, [(16, N_DESC), (1, 16)]),
            in_=_ap(t5, 0, [(80, N_DESC), (5, 16), (1, 5)]),
            axis=AX.X, op=ALU.add)


def make_inputs(types, positions, angular_neighbors, c_table):
    types = np.asarray(types).astype(np.int64)
    positions = np.ascontiguousarray(np.asarray(positions, dtype=np.float32))
    nbr = np.asarray(angular_neighbors).astype(np.int64)
    c_table = np.asarray(c_table, dtype=np.float32)
    import ml_dtypes

    pad = NTOT - N_ATOMS
    types_pad = np.concatenate([types, np.repeat(types[-1:], pad, 0)], 0)
    pos_pad = np.concatenate([positions, np.repeat(positions[-1:], pad, 0)], 0)
    nbr_pad = np.concatenate([nbr, np.repeat(nbr[-1:], pad, 0)], 0)

    # replicated u16 position table
    tbl_rec = np.zeros((NTOT, D), dtype=np.uint16)
    tbl_rec[:, :3] = np.round(pos_pad * POS_SCALE).astype(np.uint16)
    tbl = np.ascontiguousarray(
        np.broadcast_to(tbl_rec.reshape(1, NTOT * D), (P, NTOT * D)))

    # per-(t_i,t_j) c rows in [d, k] order, sqrt(0.5)-scaled
    c16 = (c_table * math.sqrt(0.5)).astype(
        ml_dtypes.bfloat16)              # [4, 4, 8, 8]

    pvec = np.arange(P)
    svec = np.arange(S)
    # atom(core, p, s) = core*CA + p*S + s
    in_maps = []
    for c in range(NCORES):
        atom = c * CA + pvec[:, None] * S + svec[None, :]      # [P, S]
        ctr = pos_pad[atom].reshape(P, S * 3).astype(np.float32)
        nbrs = nbr_pad[atom]                                    # [P, S, 20]
        tj = types_pad[nbrs]                                    # [P, S, 20]
        cj = c16[types_pad[atom][:, :, None], tj].reshape(P, PAIRS * 64)

        # gather index tile: per chunk ch (SL slots), group g:
        # n = r*CH + sl*20 + j  ->  row 16g + n%16, col col0 + n//16
        gi = np.zeros((P, PAIRS), dtype=np.int16)
        col0 = 0
        s_base = 0
        for SL in SLOT_CHUNKS:
            CH = SL * M_NBR
            for g in range(8):
                # vals[n] for n = r*CH + sl*20 + j
                vals = nbrs[16 * g:16 * g + 16,
                            s_base:s_base + SL, :].reshape(16 * CH)
                n = np.arange(16 * CH)
                gi[16 * g + (n % 16), col0 + (n // 16)] = vals
            col0 += CH
            s_base += SL
        in_maps.append({
            "tbl": tbl,
            "ctr": np.ascontiguousarray(ctr),
            "cj": np.ascontiguousarray(cj),
            "gi": np.ascontiguousarray(gi),
        })
    return in_maps


_NC_CACHE = None


def kernel(types, positions, angular_neighbors, c_table):
    global _NC_CACHE
    in_maps = make_inputs(types, positions, angular_neighbors, c_table)
    if _NC_CACHE is None:
        _NC_CACHE = build_nc()
    res = run_bass_kernel_spmd(_NC_CACHE, in_maps,
                               core_ids=list(range(NCORES)))
    outs = [res.results[c]["out"].reshape(CA, N_DESC, L_MAX)
            for c in range(NCORES)]
    q = np.concatenate(outs, 0)[:N_ATOMS]
    return np.ascontiguousarray(q)


if __name__ == "__main__":
    z = np.load("/tmp/ref_cache.npz")
    inputs = {k: z[k] for k in
              ("types", "positions", "angular_neighbors", "c_table")}
    exp = z["exp"]
    act = kernel(**inputs)
    rel = np.linalg.norm(act - exp) / np.linalg.norm(exp)
    print("Relative error:", rel)



# revision 7
# speedup vs baseline: 2.1004x; 2.1004x over previous
"""Trainium2 Bass kernel v3 for nn_AngularDescriptor (gnn_message_passing).

Legendre-addition-theorem factorization: q[i,d,l] = sum_{m in shell l}
A[i,d,m]^2 - B[i,d] with A = sum_j g_ij Y_m(u_ij), B = sum_j g_ij^2,
g scaled by sqrt(0.5) via the host-marshaled c rows.

v3 strategy versus v2: all index gathers (neighbor positions, c_table
rows) happen on the host, so the device kernel is a pure SIMD pipeline:
no GPSIMD ap_gather, no replicated position table, no per-partition
extract DMAs.  Every vector op runs full-width over all S*M = 200 pairs
per partition to amortize the ~58-cycle DVE instruction overhead.
Engines: DVE main pipeline, ACT for sqrt/cos (one table load each),
GPSIMD for the radial-basis writes and g^2.
"""
import os
import sys

sys.path.insert(0, "/opt/trn_rl_repo")
os.environ.setdefault("NEURON_RT_RESET_CORES", "1")

import math
import numpy as np

from concourse import bacc, bass, mybir, tile
from concourse.bass_utils import run_bass_kernel_spmd

# problem constants
N_ATOMS = 10000
M_NBR = 20
N_TYPES = 4
N_DESC = 8
K_MAX = 8
L_MAX = 4
R_C = 5.0

NCORES = 8
P = 128
S = 10                      # atom slots per partition
CA = P * S                  # atoms per core = 1280
NTOT = NCORES * CA          # padded atom count = 10240
PAIRS = S * M_NBR           # 200 pairs per partition

F32 = mybir.dt.float32
BF16 = mybir.dt.bfloat16

SQ3 = math.sqrt(3.0)
C31 = math.sqrt(3.0 / 8.0)
C32 = math.sqrt(15.0)
C33 = math.sqrt(5.0 / 8.0)
SHELL_OFF = [0, 1, 4, 9, 16]

AF = mybir.ActivationFunctionType
ALU = mybir.AluOpType
AX = mybir.AxisListType


def _ap(t, off, dims):
    base = t[:]
    ap = [list(base.ap[0])] + [[s, c] for (s, c) in dims]
    return bass.AP(base.tensor, base.offset + off, ap)


def build_nc(debug=False):
    nc = bacc.Bacc()
    posj_d = nc.declare_dram_parameter("posj", [P, PAIRS * 3], F32,
                                       isOutput=False)
    ctr_d = nc.declare_dram_parameter("ctr", [P, S * 3], F32, isOutput=False)
    cj_d = nc.declare_dram_parameter("cj", [P, PAIRS * 64], BF16,
                                     isOutput=False)
    out_d = nc.declare_dram_parameter("out", [P, S * N_DESC * L_MAX], F32,
                                      isOutput=True)

    with tile.TileContext(nc) as tc:
        with tc.tile_pool(name="main", bufs=1) as pool:
            # activation biases must be const APs
            cvals = [math.pi / 2, 0.0]
            consts = pool.tile([P, len(cvals)], F32)
            for ci, cv in enumerate(cvals):
                nc.vector.memset(consts[:, ci:ci + 1], cv)
                nc.const_aps.aps[(F32, cv)] = consts[:, ci:ci + 1]

            ph1 = tc.tile_pool(name="ph1", bufs=1)
            cp = ph1.__enter__()
            cj = cp.tile([P, PAIRS * 64], BF16)
            nc.scalar.dma_start(out=cj[:], in_=cj_d[:])

            ctr = pool.tile([P, S * 3], F32)
            posj = cp.tile([P, PAIRS * 3], F32)
            nc.sync.dma_start(out=ctr[:], in_=ctr_d[:])
            nc.sync.dma_start(out=posj[:], in_=posj_d[:])

            # ---- geometry: dxyz, r2, r, 1/r, u --------------------------
            C3 = PAIRS * 3
            dxyz = cp.tile([P, C3], F32)       # [s, j, 3]
            nc.vector.tensor_tensor(
                out=dxyz[:], in0=posj[:],
                in1=_ap(ctr, 0, [(3, S), (0, M_NBR), (1, 3)]),
                op=ALU.subtract)
            sq = cp.tile([P, C3], F32)
            nc.vector.tensor_tensor(out=sq[:], in0=dxyz[:], in1=dxyz[:],
                                    op=ALU.mult)
            r2 = cp.tile([P, PAIRS], F32)
            nc.vector.tensor_tensor(out=r2[:],
                                    in0=_ap(sq, 0, [(3, PAIRS)]),
                                    in1=_ap(sq, 1, [(3, PAIRS)]),
                                    op=ALU.add)
            nc.vector.tensor_tensor(out=r2[:], in0=r2[:],
                                    in1=_ap(sq, 2, [(3, PAIRS)]),
                                    op=ALU.add)
            rr = cp.tile([P, PAIRS], F32)
            nc.scalar.sqrt(out=rr[:], in_=r2[:])
            # clamp to R_C: fc(R_C) = 0 exactly, so no separate mask
            rrc = cp.tile([P, PAIRS], F32)
            nc.vector.tensor_scalar_min(out=rrc[:], in0=rr[:], scalar1=R_C)
            cosx = cp.tile([P, PAIRS], F32)
            nc.scalar.activation(out=cosx[:], in_=rrc[:], func=AF.Sin,
                                 bias=math.pi / 2, scale=-math.pi / R_C)
            rinv = cp.tile([P, PAIRS], F32)
            nc.vector.reciprocal(out=rinv[:], in_=rr[:])
            u = cp.tile([P, C3], F32)          # [s, j, 3]
            nc.vector.tensor_tensor(out=u[:], in0=dxyz[:],
                                    in1=_ap(rinv, 0, [(1, PAIRS), (0, 3)]),
                                    op=ALU.mult)

            # ---- Chebyshev recurrence (V) -------------------------------
            tm1 = cp.tile([P, PAIRS], F32)
            nc.vector.tensor_scalar(out=tm1[:], in0=rr[:], scalar1=1.0 / R_C,
                                    scalar2=-1.0, op0=ALU.mult, op1=ALU.add)
            xc = cp.tile([P, PAIRS], F32)
            nc.vector.scalar_tensor_tensor(out=xc[:], in0=tm1[:], scalar=2.0,
                                           in1=tm1[:], op0=ALU.mult,
                                           op1=ALU.mult)
            nc.vector.tensor_scalar(out=xc[:], in0=xc[:], scalar1=-1.0,
                                    scalar2=None, op0=ALU.add)
            x2 = cp.tile([P, PAIRS], F32)
            nc.vector.tensor_scalar_mul(out=x2[:], in0=xc[:], scalar1=2.0)
            T = {1: xc}
            for k in range(2, K_MAX):
                Tk = cp.tile([P, PAIRS], F32, tag=f"T{k}", name=f"T{k}")
                nc.vector.tensor_tensor(out=Tk[:], in0=x2[:], in1=T[k - 1][:],
                                        op=ALU.mult)
                if k == 2:
                    nc.vector.tensor_scalar(out=Tk[:], in0=Tk[:], scalar1=-1.0,
                                            scalar2=None, op0=ALU.add)
                else:
                    nc.vector.tensor_tensor(out=Tk[:], in0=Tk[:],
                                            in1=T[k - 2][:], op=ALU.subtract)
                T[k] = Tk

            # fch = 0.25*cosx + 0.25  (= 0.5 * fc)
            fch = cp.tile([P, PAIRS], F32)
            nc.vector.tensor_scalar(out=fch[:], in0=cosx[:], scalar1=0.25,
                                    scalar2=0.25, op0=ALU.mult, op1=ALU.add)

            # ---- radial basis ------------------------------------------
            # f[s,j,0] = fch, f[s,j,k] = T_k * fch (k>=1); the "+1" of
            # (T_k + 1) * fch is folded into c column 0 on the host.
            # GPSIMD only supports TENSOR_TENSOR, so the k=0 slice is
            # written by V and the products by G.
            f = pool.tile([P, PAIRS * K_MAX], BF16)

            def f_slice(k):
                return _ap(f, k, [(M_NBR * K_MAX, S), (K_MAX, M_NBR)])

            with nc.allow_low_precision(reason="bf16 radial basis"):
                nc.vector.tensor_scalar(out=f_slice(0), in0=cosx[:],
                                        scalar1=0.25, scalar2=0.25,
                                        op0=ALU.mult, op1=ALU.add)
                for k in range(1, K_MAX):
                    nc.gpsimd.tensor_tensor(out=f_slice(k), in0=T[k][:],
                                            in1=fch[:], op=ALU.mult)

            # ---- spherical harmonics Y[s,m,j] (V) -----------------------
            Y = pool.tile([P, PAIRS * 16], BF16)

            def y_slice(m, cnt=1):
                if cnt == 1:
                    return _ap(Y, m * M_NBR, [(16 * M_NBR, S), (1, M_NBR)])
                return _ap(Y, m * M_NBR,
                           [(16 * M_NBR, S), (M_NBR, cnt), (1, M_NBR)])

            def u_c(c):
                return _ap(u, c, [(3 * M_NBR, S), (3, M_NBR)])

            sc = cp.tile([P, PAIRS * 6], F32)

            def sc_t(i):
                return _ap(sc, i * PAIRS, [(1, PAIRS)])

            x2c, y2c, z2c, xyc, dxyc, tl = [sc_t(i) for i in range(6)]
            with nc.allow_low_precision(reason="bf16 Y"):
                nc.vector.memset(y_slice(0), 1.0)
                nc.vector.tensor_copy(
                    out=y_slice(1, 3),
                    in_=_ap(u, 0, [(3 * M_NBR, S), (1, 3), (3, M_NBR)]))
                nc.vector.tensor_tensor(out=x2c, in0=u_c(0), in1=u_c(0),
                                        op=ALU.mult)
                nc.vector.tensor_tensor(out=y2c, in0=u_c(1), in1=u_c(1),
                                        op=ALU.mult)
                nc.vector.tensor_tensor(out=z2c, in0=u_c(2), in1=u_c(2),
                                        op=ALU.mult)
                nc.vector.tensor_tensor(out=xyc, in0=u_c(0), in1=u_c(1),
                                        op=ALU.mult)
                nc.vector.tensor_scalar_mul(out=y_slice(4), in0=xyc,
                                            scalar1=SQ3)
                nc.vector.scalar_tensor_tensor(out=y_slice(5), in0=u_c(1),
                                               scalar=SQ3, in1=u_c(2),
                                               op0=ALU.mult, op1=ALU.mult)
                nc.vector.scalar_tensor_tensor(out=y_slice(6), in0=u_c(0),
                                               scalar=SQ3, in1=u_c(2),
                                               op0=ALU.mult, op1=ALU.mult)
                nc.vector.tensor_scalar(out=y_slice(7), in0=z2c, scalar1=1.5,
                                        scalar2=-0.5, op0=ALU.mult,
                                        op1=ALU.add)
                nc.vector.tensor_tensor(out=dxyc, in0=x2c, in1=y2c,
                                        op=ALU.subtract)
                nc.vector.tensor_scalar_mul(out=y_slice(8), in0=dxyc,
                                            scalar1=SQ3 / 2)
                nc.vector.tensor_scalar(out=tl, in0=z2c, scalar1=2.5,
                                        scalar2=-1.5, op0=ALU.mult,
                                        op1=ALU.add)
                nc.vector.tensor_tensor(out=y_slice(9), in0=tl, in1=u_c(2),
                                        op=ALU.mult)
                nc.vector.tensor_scalar(out=tl, in0=z2c, scalar1=5.0 * C31,
                                        scalar2=-C31, op0=ALU.mult,
                                        op1=ALU.add)
                nc.vector.tensor_tensor(out=y_slice(10), in0=tl, in1=u_c(0),
                                        op=ALU.mult)
                nc.vector.tensor_tensor(out=y_slice(11), in0=tl, in1=u_c(1),
                                        op=ALU.mult)
                nc.vector.scalar_tensor_tensor(out=y_slice(12), in0=dxyc,
                                               scalar=C32 / 2, in1=u_c(2),
                                               op0=ALU.mult, op1=ALU.mult)
                nc.vector.scalar_tensor_tensor(out=y_slice(13), in0=xyc,
                                               scalar=C32, in1=u_c(2),
                                               op0=ALU.mult, op1=ALU.mult)
                nc.vector.scalar_tensor_tensor(out=tl, in0=y2c, scalar=3.0,
                                               in1=x2c, op0=ALU.mult,
                                               op1=ALU.subtract)
                nc.vector.scalar_tensor_tensor(out=y_slice(14), in0=tl,
                                               scalar=-C33, in1=u_c(0),
                                               op0=ALU.mult, op1=ALU.mult)
                nc.vector.scalar_tensor_tensor(out=tl, in0=x2c, scalar=3.0,
                                               in1=y2c, op0=ALU.mult,
                                               op1=ALU.subtract)
                nc.vector.scalar_tensor_tensor(out=y_slice(15), in0=tl,
                                               scalar=C33, in1=u_c(1),
                                               op0=ALU.mult, op1=ALU.mult)

            # ---- g[s,d,j] = sum_k cj[s,j,d,k] * f[s,j,k]  (V) -----------
            x2t = cp.tile([P, PAIRS * 64], BF16)     # [s, j, d, k]
            a4 = cp.tile([P, PAIRS * 32], BF16)      # [s, j, d, 4]
            a2 = cp.tile([P, PAIRS * 16], BF16)      # [s, j, d, 2]
            g = pool.tile([P, PAIRS * N_DESC], BF16)   # [s, d, j]
            with nc.allow_low_precision(reason="bf16 contraction"):
                nc.vector.tensor_tensor(
                    out=_ap(x2t, 0, [(1280, S), (64, M_NBR), (8, N_DESC),
                                     (1, 8)]),
                    in0=_ap(cj, 0, [(1280, S), (64, M_NBR), (8, N_DESC),
                                    (1, 8)]),
                    in1=_ap(f, 0, [(160, S), (8, M_NBR), (0, N_DESC),
                                   (1, 8)]),
                    op=ALU.mult)
                nc.vector.tensor_tensor(
                    out=_ap(a4, 0, [(640, S), (32, M_NBR), (4, N_DESC),
                                    (1, 4)]),
                    in0=_ap(x2t, 0, [(1280, S), (64, M_NBR), (8, N_DESC),
                                     (1, 4)]),
                    in1=_ap(x2t, 4, [(1280, S), (64, M_NBR), (8, N_DESC),
                                     (1, 4)]),
                    op=ALU.add)
                nc.vector.tensor_tensor(
                    out=_ap(a2, 0, [(320, S), (16, M_NBR), (2, N_DESC),
                                    (1, 2)]),
                    in0=_ap(a4, 0, [(640, S), (32, M_NBR), (4, N_DESC),
                                    (1, 2)]),
                    in1=_ap(a4, 2, [(640, S), (32, M_NBR), (4, N_DESC),
                                    (1, 2)]),
                    op=ALU.add)
                nc.vector.tensor_tensor(
                    out=_ap(g, 0, [(160, S), (1, M_NBR), (M_NBR, N_DESC)]),
                    in0=_ap(a2, 0, [(320, S), (16, M_NBR), (2, N_DESC)]),
                    in1=_ap(a2, 1, [(320, S), (16, M_NBR), (2, N_DESC)]),
                    op=ALU.add)

            # ---- B[s,d] = sum_j g^2  (gsq on GPSIMD, reduce on V) -------
            gsq = cp.tile([P, PAIRS * N_DESC], F32)  # [s, d, j]
            gdims = [(160, S), (20, N_DESC), (1, M_NBR)]
            nc.gpsimd.tensor_tensor(out=_ap(gsq, 0, gdims),
                                    in0=_ap(g, 0, gdims),
                                    in1=_ap(g, 0, gdims), op=ALU.mult)
            Bh = pool.tile([P, S * N_DESC], F32)       # [s, d]
            nc.vector.tensor_reduce(
                out=_ap(Bh, 0, [(N_DESC, S), (1, N_DESC)]),
                in_=_ap(gsq, 0, gdims), axis=AX.X, op=ALU.add)
            ph1.__exit__(None, None, None)
            ph2 = tc.tile_pool(name="ph2", bufs=1)
            p2 = ph2.__enter__()

            # ---- A[s,d,m] = sum_j g * Y  (V tree over j = 8+8+4) --------
            xa = p2.tile([P, PAIRS * 128], BF16)     # [s, d, m, j]
            t8 = p2.tile([P, S * 128 * 8], BF16)     # [s, d, m, 8]
            t4 = p2.tile([P, S * 128 * 4], BF16)     # [s, d, m, 4]
            t4b = p2.tile([P, S * 128 * 4], BF16)
            t2 = p2.tile([P, S * 128 * 2], BF16)
            A = pool.tile([P, S * 128], F32)           # [s, d, m]
            with nc.allow_low_precision(reason="bf16 outer product"):
                nc.vector.tensor_tensor(
                    out=_ap(xa, 0, [(2560, S), (320, N_DESC), (20, 16),
                                    (1, 20)]),
                    in0=_ap(g, 0, [(160, S), (20, N_DESC), (0, 16), (1, 20)]),
                    in1=_ap(Y, 0, [(320, S), (0, N_DESC), (20, 16), (1, 20)]),
                    op=ALU.mult)
                nc.vector.tensor_tensor(
                    out=_ap(t8, 0, [(1024, S), (128, N_DESC), (8, 16),
                                    (1, 8)]),
                    in0=_ap(xa, 0, [(2560, S), (320, N_DESC), (20, 16),
                                    (1, 8)]),
                    in1=_ap(xa, 8, [(2560, S), (320, N_DESC), (20, 16),
                                    (1, 8)]),
                    op=ALU.add)
                nc.vector.tensor_tensor(
                    out=_ap(t4, 0, [(512, S), (64, N_DESC), (4, 16), (1, 4)]),
                    in0=_ap(t8, 0, [(1024, S), (128, N_DESC), (8, 16),
                                    (1, 4)]),
                    in1=_ap(t8, 4, [(1024, S), (128, N_DESC), (8, 16),
                                    (1, 4)]),
                    op=ALU.add)
                nc.vector.tensor_tensor(
                    out=_ap(t4b, 0, [(512, S), (64, N_DESC), (4, 16),
                                     (1, 4)]),
                    in0=_ap(t4, 0, [(512, S), (64, N_DESC), (4, 16), (1, 4)]),
                    in1=_ap(xa, 16, [(2560, S), (320, N_DESC), (20, 16),
                                     (1, 4)]),
                    op=ALU.add)
                nc.vector.tensor_tensor(
                    out=_ap(t2, 0, [(256, S), (32, N_DESC), (2, 16), (1, 2)]),
                    in0=_ap(t4b, 0, [(512, S), (64, N_DESC), (4, 16),
                                     (1, 2)]),
                    in1=_ap(t4b, 2, [(512, S), (64, N_DESC), (4, 16),
                                     (1, 2)]),
                    op=ALU.add)
            nc.vector.tensor_tensor(
                out=_ap(A, 0, [(128, S), (16, N_DESC), (1, 16)]),
                in0=_ap(t2, 0, [(256, S), (32, N_DESC), (2, 16)]),
                in1=_ap(t2, 1, [(256, S), (32, N_DESC), (2, 16)]),
                op=ALU.add)

            ph2.__exit__(None, None, None)

            if debug:
                for nm, t, dt in [("f", f, BF16), ("Y", Y, BF16),
                                  ("g", g, BF16), ("A", A, F32),
                                  ("Bh", Bh, F32)]:
                    dd = nc.declare_dram_parameter(
                        "d_" + nm, [P, t.shape[1]], dt, isOutput=True)
                    nc.sync.dma_start(out=dd[:], in_=t[:])

            # ---- q[s,d,l] = sum_{m in shell l} A^2 - B ------------------
            Asq = pool.tile([P, S * 128], F32)
            nc.vector.tensor_tensor(out=Asq[:], in0=A[:], in1=A[:],
                                    op=ALU.mult)
            outq = pool.tile([P, S * N_DESC * L_MAX], F32)
            q2l = pool.tile([P, S * N_DESC], F32)
            for l in range(L_MAX):
                cnt = SHELL_OFF[l + 1] - SHELL_OFF[l]
                nc.vector.tensor_reduce(
                    out=q2l[:],
                    in_=_ap(Asq, SHELL_OFF[l],
                            [(N_DESC * 16, S), (16, N_DESC), (1, cnt)]),
                    axis=AX.X, op=ALU.add)
                nc.vector.tensor_tensor(
                    out=_ap(outq, l, [(N_DESC * L_MAX, S), (L_MAX, N_DESC)]),
                    in0=q2l[:], in1=Bh[:], op=ALU.subtract)

            nc.sync.dma_start(out=out_d[:], in_=outq[:])
    nc.finalize()
    return nc


def make_inputs(types, positions, angular_neighbors, c_table):
    types = np.asarray(types).astype(np.int64)
    positions = np.ascontiguousarray(np.asarray(positions, dtype=np.float32))
    nbr = np.asarray(angular_neighbors).astype(np.int64)
    c_table = np.asarray(c_table, dtype=np.float32)
    import ml_dtypes

    pad = NTOT - N_ATOMS
    types_pad = np.concatenate([types, np.repeat(types[-1:], pad, 0)], 0)
    pos_pad = np.concatenate([positions, np.repeat(positions[-1:], pad, 0)],
                             0)
    nbr_pad = np.concatenate([nbr, np.repeat(nbr[-1:], pad, 0)], 0)

    # per-(t_i,t_j) c rows in [d, k] order, sqrt(0.5)-scaled.
    # Column 0 absorbs sum_k c[d,k] (device basis is [fch, T_1*fch, ...]).
    c_adj = c_table.astype(np.float64).copy()
    c_adj[..., 0] += c_table.astype(np.float64).sum(-1)
    c16 = (c_adj * math.sqrt(0.5)).astype(ml_dtypes.bfloat16)  # [4,4,8,8]

    pvec = np.arange(P)
    svec = np.arange(S)
    in_maps = []
    for c in range(NCORES):
        atom = c * CA + pvec[:, None] * S + svec[None, :]       # [P, S]
        nbrs = nbr_pad[atom]                                    # [P, S, 20]
        ctr = pos_pad[atom].reshape(P, S * 3).astype(np.float32)
        posj = pos_pad[nbrs].reshape(P, PAIRS * 3).astype(np.float32)
        tj = types_pad[nbrs]                                    # [P, S, 20]
        cj = c16[types_pad[atom][:, :, None], tj].reshape(P, PAIRS * 64)
        in_maps.append({
            "posj": np.ascontiguousarray(posj),
            "ctr": np.ascontiguousarray(ctr),
            "cj": np.ascontiguousarray(cj),
        })
    return in_maps


_NC_CACHE = None


def kernel(types, positions, angular_neighbors, c_table):
    global _NC_CACHE
    in_maps = make_inputs(types, positions, angular_neighbors, c_table)
    if _NC_CACHE is None:
        _NC_CACHE = build_nc()
    res = run_bass_kernel_spmd(_NC_CACHE, in_maps,
                               core_ids=list(range(NCORES)))
    outs = [res.results[c]["out"].reshape(CA, N_DESC, L_MAX)
            for c in range(NCORES)]
    q = np.concatenate(outs, 0)[:N_ATOMS]
    return np.ascontiguousarray(q.astype(np.float32))


if __name__ == "__main__":
    z = np.load("/tmp/ref_cache.npz")
    inputs = {k: z[k] for k in
              ("types", "positions", "angular_neighbors", "c_table")}
    exp = z["exp"]
    act = kernel(**inputs)
    rel = np.linalg.norm(act - exp) / np.linalg.norm(exp)
    print("Relative error:", rel)
